# revision 22
# baseline (speedup 1.0000x reference)
"""Fused pre-norm transformer block on 8 Trainium2 NeuronCores.

Problem: x[4,1024,1024]; LN -> qkv attention (16 heads) -> proj + residual;
LN -> fc1 -> gelu -> fc2 + residual.  dense_transformer, compute regime.

Sharding (no collectives): 2 cores per batch element, each owning 512 rows.
Each core redundantly computes K/V for its whole batch (key order is
softmax-invariant), so attention, MLP and both residuals are fully
core-local.  The host passes each core its batch's rows with the core's own
512 rows first; outputs are reassembled on host.

Numerics: the QK chain runs in float16 (11-bit mantissa, same as TF32, but
streams the PE at full 2-byte rate where f32r measured ~2-3x slower);
V / attn / proj / MLP run in bf16 with fp32 PSUM accumulation.  LN rsqrt is
refined with Newton steps.  All PE transposes are 16-bit (f32 transposes
measured ~3.5x slower).

Attention layout: scores are computed transposed ([keys, rows]) so exp on
ScalarE writes bf16 attn^T directly in the layout the PV matmul consumes.
The softmax shift (exact per-row max, from a separate QK pass in [rows,
keys] orientation reduced on VectorE) is folded into the scores^T matmul
as a 65th contraction row (ones on the K side, -max on the Q side) keeping
all scores^T matmuls a uniform K=65 fp16 stream.  The shift value is
softmax-invariant so fp16 rounding of the max cancels exactly.  The softmax
denominator comes from an appended ones-column of V; its reciprocal is
broadcast via a DRAM scratch bounce (partition-stride-0 reads are only
legal from DRAM).

Engine balance: LN stats stay on VectorE; PSUM->SBUF transpose drains and
residual-copy casts go to the (otherwise idle) Pool engine; biases and all
activations go to ScalarE, grouped so the 1.3us activation-table reloads
only happen at phase boundaries.  fc1/fc2 are software-pipelined with a
one-ff lag so the PE never waits on the gelu; scores/PV alternate heads
with a one-head lag so the PE never waits on the exp.
"""

import numpy as np
import ml_dtypes
from contextlib import ExitStack

import concourse.bass as bass
import concourse.tile as tile
from concourse import mybir
from concourse.bass_utils import run_bass_kernel_spmd
from concourse.vector_clock import ScopedClock, VectorClock
from concourse.masks import make_identity

F32 = mybir.dt.float32
F16 = mybir.dt.float16
BF16 = mybir.dt.bfloat16
AF = mybir.ActivationFunctionType
OP = mybir.AluOpType
AX = mybir.AxisListType

B, N, C = 4, 1024, 1024
HEADS, DK = 16, 64
DFF = 4096
EPS = 1e-5
NB = 1024   # rows per core's batch (attention keys)
NO = 512    # rows owned per core
P = 128
CCH = C // P      # 8 chunks over C
MCH = NB // P     # 8 key-row chunks
OCH = NO // P     # 4 own-row chunks
FFCH = DFF // P   # 32
NHALF = NB // 512


class SplitDrainTileContext(tile.TileContext):
    """This walrus build rejects >2 sync waits on the tail SP drain
    ("Too many sync wait commands"); split the global-clock waits across
    single-wait drain instructions."""

    def _drain_and_barrier(self, tick_clock, wait_clock):
        nc = self.nc
        gc = tick_clock.global_clock
        n = len(gc)
        for i in range(n):
            if gc[i] > 0:
                vc = VectorClock([0] * n)
                vc.require_at_least(i, gc[i])
                d = nc.sync.drain()
                wait_clock.add_sem_waits(d.ins, ScopedClock({None: vc}))
        nc.sync.drain()
        nc.all_engine_barrier()
        popped = nc._tile_sem_poison_stack.pop()
        assert popped is self._sem_poison
        nc.clear_and_free_semaphores(list(self.sems.allocated().values()))
        nc.all_engine_barrier()


def legalize_waits(nc, cap=1):
    """Walrus here allows at most one sync wait per regular instruction.
    Hoist excess waits onto same-engine NoOps placed just before."""
    n = [0]

    def mknop(engine, wait):
        n[0] += 1
        nop = mybir.InstNoOp(name=f"I-waitfix-{n[0]}", ins=[], outs=[])
        nop.engine = engine
        nop.sync_info = mybir.SyncInfo(on_wait=[wait], on_update=[])
        return nop

    for f in nc.m.functions:
        for bb in f.blocks:
            out = []
            for inst in bb.instructions:
                w = list(inst.sync_info.on_wait or []) if inst.sync_info else []
                if len(w) > cap:
                    for extra in w[:-cap]:
                        out.append(mknop(inst.engine, extra))
                    inst.sync_info.on_wait = w[-cap:]
                out.append(inst)
            bb.instructions = out


def _rsqrt(nc, pool, var, eps, newton=2):
    """[128,1] fp32: 1/sqrt(var+eps); ACT-sqrt seed + Newton steps."""
    a = pool.tile([P, 1], F32, tag="rsq_a")
    nc.vector.tensor_scalar_add(a[:], var, eps)
    s0 = pool.tile([P, 1], F32, tag="rsq_s")
    nc.scalar.activation(s0[:], a[:], AF.Sqrt)
    r = pool.tile([P, 1], F32, tag="rsq_r")
    nc.vector.reciprocal(r[:], s0[:])
    t = pool.tile([P, 1], F32, tag="rsq_t")
    for _ in range(newton):
        nc.vector.tensor_mul(t[:], r[:], r[:])
        nc.vector.tensor_mul(t[:], t[:], a[:])
        nc.vector.tensor_scalar(t[:], t[:], -0.5, 1.5, op0=OP.mult, op1=OP.add)
        nc.vector.tensor_mul(r[:], r[:], t[:])
    return r


def _layer_norm_chunk(nc, pool, x_i, xn_i, newton=2):
    """LN of one [128, C] row-chunk: xn_i = (x - mean(x)) * rsqrt(var+eps).
    Stats/rsqrt on VectorE (+one ScalarE sqrt); the big apply on the
    otherwise-idle Pool engine (SBUF-only, which Pool can reach)."""
    nsub = C // 512
    stats = pool.tile([P, nsub, 6], F32, tag="ln_stats")
    for s in range(nsub):
        nc.vector.bn_stats(stats[:, s, :], x_i[:, s * 512:(s + 1) * 512])
    mv = pool.tile([P, 2], F32, tag="ln_mv")
    nc.vector.bn_aggr(mv[:], stats[:])
    r = _rsqrt(nc, pool, mv[:, 1:2], EPS, newton=newton)
    nc.gpsimd.tensor_scalar(
        xn_i[:], x_i[:], mv[:, 0:1], r[:], op0=OP.subtract, op1=OP.mult
    )


def build_program(legalize=True):
    nc = bass.Bass()

    x = nc.declare_dram_parameter("x", [NB, C], F32, isOutput=False)
    w_qk = nc.declare_dram_parameter("w_qk", [C, 2 * C], F16, isOutput=False)
    w_v = nc.declare_dram_parameter("w_v", [C, C], F16, isOutput=False)
    w_proj = nc.declare_dram_parameter("w_proj", [C, C], BF16, isOutput=False)
    w_fc1 = nc.declare_dram_parameter("w_fc1", [C, DFF], BF16, isOutput=False)
    w_fc2 = nc.declare_dram_parameter("w_fc2", [DFF, C], BF16, isOutput=False)
    b_qk = nc.declare_dram_parameter("b_qk", [2 * C], F32, isOutput=False)
    b_v = nc.declare_dram_parameter("b_v", [C], F32, isOutput=False)
    b_proj = nc.declare_dram_parameter("b_proj", [C], F32, isOutput=False)
    b_fc1 = nc.declare_dram_parameter("b_fc1", [DFF], F32, isOutput=False)
    b_fc2 = nc.declare_dram_parameter("b_fc2", [C], F32, isOutput=False)
    out = nc.declare_dram_parameter("out", [NO, C], F32, isOutput=True)

    with SplitDrainTileContext(nc) as tc:
        with ExitStack() as ctx:
            _build_body(
                nc, tc, ctx,
                x, w_qk, w_v, w_proj, w_fc1, w_fc2,
                b_qk, b_v, b_proj, b_fc1, b_fc2, out,
            )
    if legalize:
        legalize_waits(nc)
    return nc


def _build_body(nc, tc, ctx, x, w_qk, w_v, w_proj, w_fc1, w_fc2,
                b_qk, b_v, b_proj, b_fc1, b_fc2, out):
    perm = ctx.enter_context(tc.tile_pool(name="perm", bufs=1))
    small = ctx.enter_context(tc.tile_pool(name="small", bufs=3))

    # --- constants / biases ------------------------------------------------
    ident16 = perm.tile([P, P], F16)
    make_identity(nc, ident16[:])
    identbf = perm.tile([P, P], BF16)
    make_identity(nc, identbf[:])

    bqk_sb = perm.tile([P, 2 * CCH], F32)
    nc.sync.dma_start(bqk_sb[:], b_qk.rearrange("(c p) -> p c", p=P))
    bproj_sb = perm.tile([P, CCH], F32)
    nc.sync.dma_start(bproj_sb[:], b_proj.rearrange("(c p) -> p c", p=P))
    bfc1_sb = perm.tile([P, FFCH], F32)
    nc.sync.dma_start(bfc1_sb[:], b_fc1.rearrange("(c p) -> p c", p=P))
    bfc2_sb = perm.tile([P, CCH], F32)
    nc.sync.dma_start(bfc2_sb[:], b_fc2.rearrange("(c p) -> p c", p=P))
    bv_bc = perm.tile([P, C], F32)
    nc.gpsimd.dma_start(
        bv_bc[:], bass.AP(tensor=b_v[:].tensor, offset=b_v[:].offset, ap=[[0, P], [1, C]])
    )

    x_own = perm.tile([P, OCH, C], BF16)      # own rows (residual 1)
    x2 = perm.tile([P, OCH, C], BF16)         # post-attn residual stream

    with ExitStack() as kqv_scope:
        kqv = kqv_scope.enter_context(tc.tile_pool(name="kqv", bufs=1))
        kT = kqv.tile([P, CCH, NB], F16)        # K^T head-pairs [128=2*dk, m]
        qT = kqv.tile([P, CCH, NO], F16)        # Q^T head-pairs [128=2*dk, n_own]
        v_sb = kqv.tile([P, MCH, HEADS, DK + 1], BF16)  # V rows + ones col
        ctxT = kqv.tile([P, CCH, NO], BF16)     # (attn@V)^T, normalized
        stage2 = kqv.tile([HEADS * OCH, P], F16)  # -rowmax per head, transposed
        wp_pool = kqv_scope.enter_context(tc.tile_pool(name="wp", bufs=1))
        wp_sb = wp_pool.tile([P, CCH, C], BF16)

        with ExitStack() as ph_a:
            # ============ S1: load x, LN1, transpose (pipelined) ===========
            # x chunks get a deep pool so their DMAs all issue up front and
            # never queue behind weight prefetches.
            xnt_pool = ph_a.enter_context(tc.tile_pool(name="xnt", bufs=1))
            xnT = xnt_pool.tile([P, CCH, NB], F16)   # LN1(x)^T  [c, n]
            wqv_pool = ph_a.enter_context(tc.tile_pool(name="wqv", bufs=1))

            with ExitStack() as s1_scope:
                ln = s1_scope.enter_context(tc.tile_pool(name="ln", bufs=4))
                xoth = s1_scope.enter_context(tc.tile_pool(name="xoth", bufs=3))
                tps = s1_scope.enter_context(
                    tc.tile_pool(name="tps", bufs=4, space="PSUM"))

                x_tiles = []
                for i in range(MCH):
                    x_i = xoth.tile([P, C], F32, tag=f"x_{i}", name=f"x_{i}",
                                    bufs=1)
                    nc.sync.dma_start(x_i[:], x[i * P:(i + 1) * P, :])
                    x_tiles.append(x_i)

                # weight prefetch behind the x chunks: qk then v
                wqk_sb = wqv_pool.tile([P, CCH, 2 * C], F16)
                wv_sb = wqv_pool.tile([P, CCH, C], F16)
                for c in range(CCH):
                    nc.sync.dma_start(wqk_sb[:, c, :], w_qk[c * P:(c + 1) * P, :])
                for c in range(CCH):
                    nc.sync.dma_start(wv_sb[:, c, :], w_v[c * P:(c + 1) * P, :])

                for i in range(MCH):
                    x_i = x_tiles[i]
                    if i < OCH:
                        nc.gpsimd.tensor_copy(x_own[:, i, :], x_i[:])
                    xn_i = xoth.tile([P, C], F16, tag="xn_i")
                    _layer_norm_chunk(nc, ln, x_i, xn_i)
                    for c in range(CCH):
                        pst = tps.tile([P, P], F16)
                        nc.tensor.transpose(pst[:], xn_i[:, c * P:(c + 1) * P],
                                            ident16[:])
                        nc.vector.tensor_copy(xnT[:, c, i * P:(i + 1) * P],
                                              pst[:])

            # ============ S2: Q^T, K^T, QK1 rowmax, V ======================
            qkps = ph_a.enter_context(tc.tile_pool(name="qkps", bufs=2, space="PSUM"))
            qk1ps = ph_a.enter_context(tc.tile_pool(name="qk1ps", bufs=2, space="PSUM"))
            pstgps = ph_a.enter_context(tc.tile_pool(name="pstgps", bufs=1, space="PSUM"))
            sm1 = ph_a.enter_context(tc.tile_pool(name="sm1", bufs=2))

            for p in range(CCH):
                ps = qkps.tile([P, NO], F32, tag="qk_ps")
                for c in range(CCH):
                    nc.tensor.matmul(ps[:], wqk_sb[:, c, p * P:(p + 1) * P],
                                     xnT[:, c, 0:NO],
                                     start=(c == 0), stop=(c == CCH - 1))
                nc.scalar.activation(qT[:, p, :], ps[:], AF.Identity,
                                     bias=bqk_sb[:, p:p + 1])
            for p in range(CCH):
                for nh in range(NHALF):
                    ps = qkps.tile([P, 512], F32, tag="qk_ps")
                    for c in range(CCH):
                        nc.tensor.matmul(
                            ps[:], wqk_sb[:, c, C + p * P:C + (p + 1) * P],
                            xnT[:, c, nh * 512:(nh + 1) * 512],
                            start=(c == 0), stop=(c == CCH - 1))
                    nc.scalar.activation(
                        kT[:, p, nh * 512:(nh + 1) * 512], ps[:], AF.Identity,
                        bias=bqk_sb[:, CCH + p:CCH + p + 1])

            # QK1 (exact per-row -max of scores, [rows, keys] orientation)
            # interleaved 2:1 with the V matmuls so the PE never waits on the
            # drain/reduce stream: ScalarE drains one head-half to fp16
            # (reduced at 2x), VectorE reduces the other straight from PSUM.
            # stg col (h*OCH + ncc) holds -max for head h, row-chunk ncc.
            stg = sm1.tile([P, HEADS * OCH], F16, name="stg")
            for mc in range(MCH):
                nc.gpsimd.memset(v_sb[:, mc, :, DK:DK + 1], 1.0)

            def _emit_qk1(pp, ncc):
                ps1a = qk1ps.tile([P, NB], F32, tag="ps1")
                ps1b = qk1ps.tile([P, NB], F32, tag="ps1")
                for mh in range(NHALF):
                    nc.tensor.matmul(
                        ps1a[:, mh * 512:(mh + 1) * 512],
                        qT[0:DK, pp, ncc * P:(ncc + 1) * P],
                        kT[0:DK, pp, mh * 512:(mh + 1) * 512],
                        start=True, stop=True)
                    nc.tensor.matmul(
                        ps1b[:, mh * 512:(mh + 1) * 512],
                        qT[DK:P, pp, ncc * P:(ncc + 1) * P],
                        kT[DK:P, pp, mh * 512:(mh + 1) * 512],
                        start=True, stop=True)
                s16a = sm1.tile([P, NB], F16, tag="s16", bufs=3)
                nc.scalar.copy(s16a[:], ps1a[:])
                ha, hb = 2 * pp, 2 * pp + 1
                nc.vector.reduce_max(
                    stg[:, ha * OCH + ncc:ha * OCH + ncc + 1], s16a[:],
                    axis=AX.X, negate=True)
                nc.vector.reduce_max(
                    stg[:, hb * OCH + ncc:hb * OCH + ncc + 1], ps1b[:],
                    axis=AX.X, negate=True)

            def _emit_v(dh, mc):
                ps = qkps.tile([P, 512], F32, tag="qk_ps")
                for c in range(CCH):
                    nc.tensor.matmul(
                        ps[:], xnT[:, c, mc * P:(mc + 1) * P],
                        wv_sb[:, c, dh * 512:(dh + 1) * 512],
                        start=(c == 0), stop=(c == CCH - 1))
                nc.vector.tensor_tensor(
                    out=v_sb[:, mc, dh * 8:(dh + 1) * 8, 0:DK],
                    in0=ps[:].rearrange("p (h d) -> p h d", d=DK),
                    in1=bv_bc[:, dh * 512:(dh + 1) * 512].rearrange(
                        "p (h d) -> p h d", d=DK),
                    op=OP.add,
                )

            v_iters = [(dh, mc) for dh in range(2) for mc in range(MCH)]
            vi = 0
            for idx, (pp, ncc) in enumerate(
                    (pp, ncc) for pp in range(CCH) for ncc in range(OCH)):
                _emit_qk1(pp, ncc)
                if idx % 2 == 1:
                    _emit_v(*v_iters[vi])
                    vi += 1
            for dh, mc in v_iters[vi:]:
                _emit_v(dh, mc)
            pstg = pstgps.tile([HEADS * OCH, P], F16, name="pstg")
            nc.tensor.transpose(pstg[:], stg[:], ident16[:])
            nc.vector.tensor_copy(stage2[:], pstg[:])

        # ================ S3: attention ===================================
        # Per-head [65, ...] K/Q tiles (row 64 = softmax-shift augmentation:
        # ones on K, -rowmax on Q) rotate through 4 manually-managed tiles;
        # their ones rows are set once.  scores(h+1) is emitted before PV(h)
        # so the PE streams through the exp latency.
        with ExitStack() as ph_b:
            kq65 = ph_b.enter_context(tc.tile_pool(name="kq65", bufs=1))
            att = ph_b.enter_context(tc.tile_pool(name="att", bufs=2))
            sm = ph_b.enter_context(tc.tile_pool(name="sm", bufs=4))
            rbc = ph_b.enter_context(tc.tile_pool(name="rbc", bufs=3))
            drp = ph_b.enter_context(tc.tile_pool(name="drp", bufs=2, space="DRAM"))
            qk2ps = ph_b.enter_context(
                tc.tile_pool(name="qk2ps", bufs=3, space="PSUM"))
            ctxps = ph_b.enter_context(
                tc.tile_pool(name="ctxps", bufs=4, space="PSUM"))
            NKQ = 4
            kts = [kq65.tile([DK + 1, NB], F16, name=f"kt65_{i}")
                   for i in range(NKQ)]
            qts = [kq65.tile([DK + 1, NO], F16, name=f"qt65_{i}")
                   for i in range(NKQ)]
            for i in range(NKQ):
                nc.gpsimd.memset(kts[i][DK:DK + 1, :], 1.0)

            # proj weight prefetch on the gpsimd DMA queue so it never delays
            # the latency-sensitive softmax-denominator bounces on the sync
            # queue
            for c in range(CCH):
                nc.gpsimd.dma_start(wp_sb[:, c, :], w_proj[c * P:(c + 1) * P, :])

            pend = {}

            def _emit_scores(h):
                half, pp, off = h // CCH, h // 2, (h % 2) * DK
                kt, qt = kts[h % NKQ], qts[h % NKQ]
                nc.vector.tensor_copy(kt[0:DK, :], kT[off:off + DK, pp, :])
                nc.vector.tensor_copy(qt[0:DK, :], qT[off:off + DK, pp, :])
                nc.sync.dma_start(qt[DK:DK + 1, :],
                                  stage2[h * OCH:(h + 1) * OCH, :])
                attnT = att.tile([P, MCH, 512], BF16, tag="attnT")
                for mc in range(MCH):
                    ps2 = qk2ps.tile([P, 512], F32, tag="ps2")
                    nc.tensor.matmul(
                        ps2[:], kt[:, mc * P:(mc + 1) * P], qt[:],
                        start=True, stop=True)
                    nc.scalar.activation(attnT[:, mc, :], ps2[:], AF.Exp)
                return attnT

            def _emit_pv(h, attnT):
                ps3 = ctxps.tile([DK + 1, 512], F32, tag="ps3")
                for mc in range(MCH):
                    nc.tensor.matmul(ps3[:], v_sb[:, mc, h, :], attnT[:, mc, :],
                                     start=(mc == 0), stop=(mc == MCH - 1))
                s_row = sm.tile([1, 512], F32, tag="s_row")
                nc.vector.tensor_copy(s_row[:], ps3[DK:DK + 1, :])
                s_scr = drp.tile([1, 512], F32, tag="s_scr")
                nc.sync.dma_start(s_scr[:], s_row[:])
                s_sq = sm.tile([P, 4], F32, tag="s_sq")
                nc.sync.dma_start(
                    s_sq[:],
                    bass.AP(tensor=s_scr.tensor, offset=s_scr.offset,
                            ap=[[4, P], [1, 4]]))
                r_sq = sm.tile([P, 4], F32, tag="r_sq")
                nc.vector.reciprocal(r_sq[:], s_sq[:])
                r_scr = drp.tile([1, 512], F32, tag="r_scr")
                nc.sync.dma_start(
                    bass.AP(tensor=r_scr.tensor, offset=r_scr.offset,
                            ap=[[4, P], [1, 4]]), r_sq[:])
                r_bc = rbc.tile([DK, 512], F32, tag="r_bc")
                nc.sync.dma_start(
                    r_bc[:],
                    bass.AP(tensor=r_scr.tensor, offset=r_scr.offset,
                            ap=[[0, DK], [1, 512]]))
                pend[h] = (ps3, r_bc)

            def _emit_norm(h):
                off2 = (h % 2) * DK
                ps3h, r_bch = pend.pop(h)
                nc.vector.tensor_tensor(
                    out=ctxT[off2:off2 + DK, h // 2, :], in0=ps3h[0:DK, :],
                    in1=r_bch[:], op=OP.mult)

            prev_at = None
            for h in range(HEADS):
                at = _emit_scores(h)
                if h >= 1:
                    _emit_pv(h - 1, prev_at)
                if h >= 2:
                    _emit_norm(h - 2)
                prev_at = at
            _emit_pv(HEADS - 1, prev_at)
            _emit_norm(HEADS - 2)
            _emit_norm(HEADS - 1)

        # ================ S4: proj + residual =============================
        with ExitStack() as ph_c:
            psb_pool = ph_c.enter_context(tc.tile_pool(name="psb", bufs=1))
            p_sb = psb_pool.tile([P, OCH, C], BF16)
            pps = ph_c.enter_context(tc.tile_pool(name="pps", bufs=2, space="PSUM"))
            tps2 = ph_c.enter_context(tc.tile_pool(name="tps2", bufs=4, space="PSUM"))
            sc = ph_c.enter_context(tc.tile_pool(name="sc", bufs=3))
            for co in range(CCH):
                ps = pps.tile([P, 512], F32)
                for c in range(CCH):
                    nc.tensor.matmul(ps[:], wp_sb[:, c, co * P:(co + 1) * P],
                                     ctxT[:, c, :],
                                     start=(c == 0), stop=(c == CCH - 1))
                pT_i = sc.tile([P, 512], BF16, tag="pT_i")
                nc.scalar.activation(pT_i[:], ps[:], AF.Identity,
                                     bias=bproj_sb[:, co:co + 1])
                for ncc in range(OCH):
                    pst = tps2.tile([P, P], BF16)
                    nc.tensor.transpose(
                        pst[:], pT_i[:, ncc * P:(ncc + 1) * P], identbf[:])
                    nc.vector.tensor_copy(p_sb[:, ncc, co * P:(co + 1) * P], pst[:])
            for i in range(OCH):
                nc.vector.tensor_tensor(
                    out=x2[:, i, :], in0=x_own[:, i, :], in1=p_sb[:, i, :], op=OP.add)

    # ================ S4b: LN2 + transpose ================================
    with ExitStack() as mlp_scope:
        mlp = mlp_scope.enter_context(tc.tile_pool(name="mlp", bufs=1))
        x2nT = mlp.tile([P, CCH, NO], BF16)
        hT = mlp.tile([P, FFCH, NO], BF16)
        wf2 = mlp_scope.enter_context(tc.tile_pool(name="wf2", bufs=1))
        wf2_t = wf2.tile([P, FFCH, C], BF16)
        wf1 = mlp_scope.enter_context(tc.tile_pool(name="wf1", bufs=1))
        wf1_t = wf1.tile([P, CCH, DFF], BF16)
        # wf1 lands ff-major in 128-col blocks on the gpsimd queue so fc1's
        # first chunk is ready ~1us in; wf2 streams on the sync queue and
        # only needs to stay ahead of the fc2 accumulation
        for ffb in range(FFCH):
            for c in range(CCH):
                nc.gpsimd.dma_start(
                    wf1_t[:, c, ffb * P:(ffb + 1) * P],
                    w_fc1[c * P:(c + 1) * P, ffb * P:(ffb + 1) * P])
        for ff in range(FFCH):
            nc.sync.dma_start(wf2_t[:, ff, :], w_fc2[ff * P:(ff + 1) * P, :])
        with ExitStack() as ph_d:
            ln2 = ph_d.enter_context(tc.tile_pool(name="ln2", bufs=4))
            tps3 = ph_d.enter_context(tc.tile_pool(name="tps3", bufs=4, space="PSUM"))
            for i in range(OCH):
                x2n_i = ln2.tile([P, C], BF16, tag="x2n_i")
                _layer_norm_chunk(nc, ln2, x2[:, i, :], x2n_i, newton=1)
                for c in range(CCH):
                    pst = tps3.tile([P, P], BF16)
                    nc.tensor.transpose(
                        pst[:], x2n_i[:, c * P:(c + 1) * P], identbf[:])
                    nc.vector.tensor_copy(x2nT[:, c, i * P:(i + 1) * P], pst[:])

        # ======== S5/S6: fc1+gelu pipelined into fc2, residual, store =====
        with ExitStack() as ph_f:
            f1ps = ph_f.enter_context(tc.tile_pool(name="f1ps", bufs=2, space="PSUM"))
            f2ps = ph_f.enter_context(tc.tile_pool(name="f2ps", bufs=1, space="PSUM"))
            tps4 = ph_f.enter_context(tc.tile_pool(name="tps4", bufs=2, space="PSUM"))
            sc2 = ph_f.enter_context(tc.tile_pool(name="sc2", bufs=3))
            o_pool = ph_f.enter_context(tc.tile_pool(name="o_pool", bufs=1))
            o_t = o_pool.tile([P, OCH, C], BF16)

            def _fc1(ff):
                ps = f1ps.tile([P, 512], F32)
                for c in range(CCH):
                    nc.tensor.matmul(
                        ps[:], wf1_t[:, c, ff * P:(ff + 1) * P], x2nT[:, c, :],
                        start=(c == 0), stop=(c == CCH - 1))
                nc.scalar.activation(hT[:, ff, :], ps[:], AF.Gelu,
                                     bias=bfc1_sb[:, ff:ff + 1])

            def _fc2(grp, psacc, ff):
                for j in range(4):
                    co = grp * 4 + j
                    nc.tensor.matmul(
                        psacc[j][:], wf2_t[:, ff, co * P:(co + 1) * P],
                        hT[:, ff, :],
                        start=(ff == 0), stop=(ff == FFCH - 1))

            def _fc2_tail(grp, psacc):
                for j in range(4):
                    co = grp * 4 + j
                    oT_i = sc2.tile([P, 512], BF16, tag="oT_i", bufs=2)
                    nc.scalar.activation(oT_i[:], psacc[j][:], AF.Identity,
                                         bias=bfc2_sb[:, co:co + 1])
                    for ncc in range(OCH):
                        pst = tps4.tile([P, P], BF16)
                        nc.tensor.transpose(
                            pst[:], oT_i[:, ncc * P:(ncc + 1) * P], identbf[:])
                        nc.vector.tensor_copy(
                            o_t[:, ncc, co * P:(co + 1) * P], pst[:])

            # group 0 rides along with fc1, one ff behind so the PE never
            # waits on the gelu
            psacc = [f2ps.tile([P, 512], F32, tag=f"f2acc{j}", name=f"f2acc{j}")
                     for j in range(4)]
            for ff in range(FFCH):
                _fc1(ff)
                if ff >= 1:
                    _fc2(0, psacc, ff - 1)
            _fc2(0, psacc, FFCH - 1)
            _fc2_tail(0, psacc)
            psacc = [f2ps.tile([P, 512], F32, tag=f"f2acc{j}", name=f"f2accB{j}")
                     for j in range(4)]
            for ff in range(FFCH):
                _fc2(1, psacc, ff)
            _fc2_tail(1, psacc)

            for i in range(OCH):
                fin = sc2.tile([P, C], F32, tag="fin", bufs=2)
                nc.vector.tensor_tensor(
                    out=fin[:], in0=x2[:, i, :], in1=o_t[:, i, :], op=OP.add)
                nc.sync.dma_start(out[i * P:(i + 1) * P, :], fin[:])


_NC_CACHE = [None]


def _get_nc():
    if _NC_CACHE[0] is None:
        _NC_CACHE[0] = build_program()
    return _NC_CACHE[0]


def _prepare_in_maps(inputs):
    f32 = lambda a: np.ascontiguousarray(np.asarray(a, dtype=np.float32))
    x = f32(inputs["x"])
    g = f32(inputs["norm_g"])
    bb = f32(inputs["norm_b"])
    w_qkv = f32(inputs["w_qkv"])
    b_qkv = f32(inputs["b_qkv"])
    w_proj = f32(inputs["w_proj"])
    b_proj = f32(inputs["b_proj"])
    w_fc1 = f32(inputs["w_fc1"])
    b_fc1 = f32(inputs["b_fc1"])
    w_fc2 = f32(inputs["w_fc2"])
    b_fc2 = f32(inputs["b_fc2"])

    # fold the LN affine into the consuming matmuls; fold the sqrt(dk)
    # score scale into w_q/b_q
    w_qkv_f = w_qkv * g[:, None]
    b_qkv_f = b_qkv + bb @ w_qkv
    scale = float(DK) ** 0.5
    w_q = w_qkv_f[:, 0:C] * scale
    b_q = b_qkv_f[0:C] * scale
    w_k = w_qkv_f[:, C:2 * C]
    b_k = b_qkv_f[C:2 * C]
    w_v = np.ascontiguousarray(w_qkv_f[:, 2 * C:3 * C])
    b_v = np.ascontiguousarray(b_qkv_f[2 * C:3 * C])
    w_fc1_f = w_fc1 * g[:, None]
    b_fc1_f = b_fc1 + bb @ w_fc1

    bf = lambda a: np.ascontiguousarray(a.astype(ml_dtypes.bfloat16))
    f16 = lambda a: np.ascontiguousarray(a.astype(np.float16))
    shared = {
        "w_qk": f16(np.concatenate([w_q, w_k], axis=1)),
        "w_v": f16(w_v),
        "w_proj": bf(w_proj),
        "w_fc1": bf(w_fc1_f),
        "w_fc2": bf(w_fc2),
        "b_qk": np.ascontiguousarray(np.concatenate([b_q, b_k])),
        "b_v": b_v,
        "b_proj": b_proj,
        "b_fc1": np.ascontiguousarray(b_fc1_f),
        "b_fc2": b_fc2,
    }
    in_maps = []
    for core in range(8):
        b, half = core // 2, core % 2
        xb = x[b]
        x_core = np.ascontiguousarray(np.concatenate(
            [xb[half * NO:(half + 1) * NO], xb[(1 - half) * NO:(2 - half) * NO]],
            axis=0))
        in_maps.append({"x": x_core, **shared})
    return in_maps


def kernel(**inputs) -> np.ndarray:
    nc = _get_nc()
    in_maps = _prepare_in_maps(inputs)
    res = run_bass_kernel_spmd(nc, in_maps, list(range(8)))
    out = np.empty((B, N, C), dtype=np.float32)
    for core in range(8):
        b, half = core // 2, core % 2
        out[b, half * NO:(half + 1) * NO] = res.results[core]["out"]
    return out


# revision 26
# speedup vs baseline: 1.7470x; 1.7470x over previous
"""Fused pre-norm transformer block on 8 Trainium2 NeuronCores.

Problem: x[4,1024,1024]; LN -> qkv attention (16 heads) -> proj + residual;
LN -> fc1 -> gelu -> fc2 + residual.  dense_transformer, compute regime.

Sharding (no collectives): 2 cores per batch element, each owning 512 rows.
Each core redundantly computes K/V for its whole batch (key order is
softmax-invariant), so attention, MLP and both residuals are fully
core-local.  The host passes each core its batch's rows with the core's own
512 rows first; outputs are reassembled on host.

Numerics: the QK chain runs in float16 (11-bit mantissa, same as TF32, but
streams the PE at full 2-byte rate where f32r measured ~2-3x slower);
V / attn / proj / MLP run in bf16 with fp32 PSUM accumulation.  LN rsqrt is
refined with Newton steps.  All PE transposes are 16-bit (f32 transposes
measured ~3.5x slower).

Attention layout: scores are computed transposed ([keys, rows]) so exp on
ScalarE writes bf16 attn^T directly in the layout the PV matmul consumes.
The softmax shift (exact per-row max, from a separate QK pass in [rows,
keys] orientation reduced on VectorE) is folded into the scores^T matmul
as a 65th contraction row (ones on the K side, -max on the Q side) keeping
all scores^T matmuls a uniform K=65 fp16 stream.  The shift value is
softmax-invariant so fp16 rounding of the max cancels exactly.  The softmax
denominator comes from an appended ones-column of V; its reciprocal is
broadcast via a DRAM scratch bounce (partition-stride-0 reads are only
legal from DRAM).

Engine balance: LN stats stay on VectorE; PSUM->SBUF transpose drains and
residual-copy casts go to the (otherwise idle) Pool engine; biases and all
activations go to ScalarE, grouped so the 1.3us activation-table reloads
only happen at phase boundaries.  fc1/fc2 are software-pipelined with a
one-ff lag so the PE never waits on the gelu; scores/PV alternate heads
with a one-head lag so the PE never waits on the exp.
"""

import numpy as np
import ml_dtypes
from contextlib import ExitStack

import concourse.bass as bass
import concourse.tile as tile
from concourse import mybir
from concourse.bass_utils import run_bass_kernel_spmd
from concourse.vector_clock import ScopedClock, VectorClock
from concourse.masks import make_identity

F32 = mybir.dt.float32
F16 = mybir.dt.float16
BF16 = mybir.dt.bfloat16
AF = mybir.ActivationFunctionType
OP = mybir.AluOpType
AX = mybir.AxisListType

B, N, C = 4, 1024, 1024
HEADS, DK = 16, 64
DFF = 4096
EPS = 1e-5
NB = 1024   # rows per core's batch (attention keys)
NO = 512    # rows owned per core
P = 128
CCH = C // P      # 8 chunks over C
MCH = NB // P     # 8 key-row chunks
OCH = NO // P     # 4 own-row chunks
FFCH = DFF // P   # 32
NHALF = NB // 512


class SplitDrainTileContext(tile.TileContext):
    """This walrus build rejects >2 sync waits on the tail SP drain
    ("Too many sync wait commands"); split the global-clock waits across
    single-wait drain instructions."""

    def _drain_and_barrier(self, tick_clock, wait_clock):
        nc = self.nc
        gc = tick_clock.global_clock
        n = len(gc)
        for i in range(n):
            if gc[i] > 0:
                vc = VectorClock([0] * n)
                vc.require_at_least(i, gc[i])
                d = nc.sync.drain()
                wait_clock.add_sem_waits(d.ins, ScopedClock({None: vc}))
        nc.sync.drain()
        nc.all_engine_barrier()
        popped = nc._tile_sem_poison_stack.pop()
        assert popped is self._sem_poison
        nc.clear_and_free_semaphores(list(self.sems.allocated().values()))
        nc.all_engine_barrier()


def legalize_waits(nc, cap=1):
    """Walrus here allows at most one sync wait per regular instruction.
    Hoist excess waits onto same-engine NoOps placed just before."""
    n = [0]

    def mknop(engine, wait):
        n[0] += 1
        nop = mybir.InstNoOp(name=f"I-waitfix-{n[0]}", ins=[], outs=[])
        nop.engine = engine
        nop.sync_info = mybir.SyncInfo(on_wait=[wait], on_update=[])
        return nop

    for f in nc.m.functions:
        for bb in f.blocks:
            out = []
            for inst in bb.instructions:
                w = list(inst.sync_info.on_wait or []) if inst.sync_info else []
                if len(w) > cap:
                    for extra in w[:-cap]:
                        out.append(mknop(inst.engine, extra))
                    inst.sync_info.on_wait = w[-cap:]
                out.append(inst)
            bb.instructions = out


def _rsqrt(nc, pool, var, eps, newton=2):
    """[128,1] fp32: 1/sqrt(var+eps); ACT-sqrt seed + Newton steps."""
    a = pool.tile([P, 1], F32, tag="rsq_a")
    nc.vector.tensor_scalar_add(a[:], var, eps)
    s0 = pool.tile([P, 1], F32, tag="rsq_s")
    nc.scalar.activation(s0[:], a[:], AF.Sqrt)
    r = pool.tile([P, 1], F32, tag="rsq_r")
    nc.vector.reciprocal(r[:], s0[:])
    t = pool.tile([P, 1], F32, tag="rsq_t")
    for _ in range(newton):
        nc.vector.tensor_mul(t[:], r[:], r[:])
        nc.vector.tensor_mul(t[:], t[:], a[:])
        nc.vector.tensor_scalar(t[:], t[:], -0.5, 1.5, op0=OP.mult, op1=OP.add)
        nc.vector.tensor_mul(r[:], r[:], t[:])
    return r


def _layer_norm_chunk(nc, pool, x_i, xn_i, newton=2):
    """LN of one [128, C] row-chunk: xn_i = (x - mean(x)) * rsqrt(var+eps)."""
    nsub = C // 512
    stats = pool.tile([P, nsub, 6], F32, tag="ln_stats")
    for s in range(nsub):
        nc.vector.bn_stats(stats[:, s, :], x_i[:, s * 512:(s + 1) * 512])
    mv = pool.tile([P, 2], F32, tag="ln_mv")
    nc.vector.bn_aggr(mv[:], stats[:])
    r = _rsqrt(nc, pool, mv[:, 1:2], EPS, newton=newton)
    nc.vector.tensor_scalar(
        xn_i[:], x_i[:], mv[:, 0:1], r[:], op0=OP.subtract, op1=OP.mult
    )


def build_program(legalize=True):
    nc = bass.Bass()

    x = nc.declare_dram_parameter("x", [NB, C], F32, isOutput=False)
    w_qk = nc.declare_dram_parameter("w_qk", [C, 2 * C], F16, isOutput=False)
    w_v = nc.declare_dram_parameter("w_v", [C, C], F16, isOutput=False)
    w_proj = nc.declare_dram_parameter("w_proj", [C, C], BF16, isOutput=False)
    w_fc1 = nc.declare_dram_parameter("w_fc1", [C, DFF], BF16, isOutput=False)
    w_fc2 = nc.declare_dram_parameter("w_fc2", [DFF, C], BF16, isOutput=False)
    b_qk = nc.declare_dram_parameter("b_qk", [2 * C], F32, isOutput=False)
    b_proj = nc.declare_dram_parameter("b_proj", [C], F32, isOutput=False)
    b_fc1 = nc.declare_dram_parameter("b_fc1", [DFF], F32, isOutput=False)
    b_fc2 = nc.declare_dram_parameter("b_fc2", [C], F32, isOutput=False)
    out = nc.declare_dram_parameter("out", [NO, C], F32, isOutput=True)

    with SplitDrainTileContext(nc) as tc:
        with ExitStack() as ctx:
            _build_body(
                nc, tc, ctx,
                x, w_qk, w_v, w_proj, w_fc1, w_fc2,
                b_qk, b_proj, b_fc1, b_fc2, out,
            )
    if legalize:
        legalize_waits(nc)
    return nc


def _build_body(nc, tc, ctx, x, w_qk, w_v, w_proj, w_fc1, w_fc2,
                b_qk, b_proj, b_fc1, b_fc2, out):
    perm = ctx.enter_context(tc.tile_pool(name="perm", bufs=1))
    small = ctx.enter_context(tc.tile_pool(name="small", bufs=3))

    # --- constants / biases ------------------------------------------------
    ident16 = perm.tile([P, P], F16)
    make_identity(nc, ident16[:])
    identbf = perm.tile([P, P], BF16)
    make_identity(nc, identbf[:])

    bqk_sb = perm.tile([P, 2 * CCH], F32)
    nc.sync.dma_start(bqk_sb[:], b_qk.rearrange("(c p) -> p c", p=P))
    bproj_sb = perm.tile([P, CCH], F32)
    nc.sync.dma_start(bproj_sb[:], b_proj.rearrange("(c p) -> p c", p=P))
    bfc1_sb = perm.tile([P, FFCH], F32)
    nc.sync.dma_start(bfc1_sb[:], b_fc1.rearrange("(c p) -> p c", p=P))
    bfc2_sb = perm.tile([P, CCH], F32)
    nc.sync.dma_start(bfc2_sb[:], b_fc2.rearrange("(c p) -> p c", p=P))
    x_own = perm.tile([P, OCH, C], BF16)      # own rows (residual 1)
    x2 = perm.tile([P, OCH, C], BF16)         # post-attn residual stream

    with ExitStack() as kqv_scope:
        kqv = kqv_scope.enter_context(tc.tile_pool(name="kqv", bufs=1))
        kT = kqv.tile([P, CCH, NB], F16)        # K^T head-pairs [128=2*dk, m]
        qT = kqv.tile([P, CCH, NO], F16)        # Q^T head-pairs [128=2*dk, n_own]
        v_sb = kqv.tile([P, MCH, HEADS, DK + 1], BF16)  # V rows + ones col
        ctxT = kqv.tile([P, CCH, NO], BF16)     # (attn@V)^T, normalized
        stage2 = kqv.tile([HEADS * OCH, P], F16)  # -rowmax per head, transposed
        wp_pool = kqv_scope.enter_context(tc.tile_pool(name="wp", bufs=1))
        wp_sb = wp_pool.tile([P, CCH, C], BF16)

        with ExitStack() as ph_a:
            # ===== S1/S2 software pipeline ================================
            # LN chunks 0-3 -> Q + K(first key half) on the PE while chunks
            # 4-7 LN on VectorE -> K(second half) -> QK1 rowmax 2:1
            # interleaved with V.  The rowmax reduces are VectorE rate-bound,
            # so everything else in that window routes to ScalarE/PE.
            xnt_pool = ph_a.enter_context(tc.tile_pool(name="xnt", bufs=1))
            xnT = xnt_pool.tile([P, CCH, NB], F16)   # LN1(x)^T  [c, n]
            wqv_pool = ph_a.enter_context(tc.tile_pool(name="wqv", bufs=1))
            qkps = ph_a.enter_context(tc.tile_pool(name="qkps", bufs=2, space="PSUM"))
            sm1 = ph_a.enter_context(tc.tile_pool(name="sm1", bufs=2))

            s1_scope = ExitStack()
            ln = s1_scope.enter_context(tc.tile_pool(name="ln", bufs=4))
            xoth = s1_scope.enter_context(tc.tile_pool(name="xoth", bufs=3))
            tps = s1_scope.enter_context(
                tc.tile_pool(name="tps", bufs=4, space="PSUM"))

            x_tiles = []
            for i in range(MCH):
                x_i = xoth.tile([P, C], F32, tag=f"x_{i}", name=f"x_{i}",
                                bufs=1)
                nc.sync.dma_start(x_i[:], x[i * P:(i + 1) * P, :])
                x_tiles.append(x_i)

            # weight prefetch behind the x chunks: qk then v
            wqk_sb = wqv_pool.tile([P, CCH, 2 * C], F16)
            wv_sb = wqv_pool.tile([P, CCH, C], F16)
            for c in range(CCH):
                nc.sync.dma_start(wqk_sb[:, c, :], w_qk[c * P:(c + 1) * P, :])
            for c in range(CCH):
                nc.sync.dma_start(wv_sb[:, c, :], w_v[c * P:(c + 1) * P, :])

            def _ln_chunk(i):
                x_i = x_tiles[i]
                if i < OCH:
                    nc.vector.tensor_copy(x_own[:, i, :], x_i[:])
                xn_i = xoth.tile([P, C], F16, tag="xn_i")
                _layer_norm_chunk(nc, ln, x_i, xn_i)
                for c in range(CCH):
                    pst = tps.tile([P, P], F16)
                    nc.tensor.transpose(pst[:], xn_i[:, c * P:(c + 1) * P],
                                        ident16[:])
                    nc.vector.tensor_copy(xnT[:, c, i * P:(i + 1) * P],
                                          pst[:])

            def _emit_k(p, nh):
                ps = qkps.tile([P, 512], F32, tag="qk_ps")
                for c in range(CCH):
                    nc.tensor.matmul(
                        ps[:], wqk_sb[:, c, C + p * P:C + (p + 1) * P],
                        xnT[:, c, nh * 512:(nh + 1) * 512],
                        start=(c == 0), stop=(c == CCH - 1))
                nc.scalar.activation(
                    kT[:, p, nh * 512:(nh + 1) * 512], ps[:], AF.Identity,
                    bias=bqk_sb[:, CCH + p:CCH + p + 1])

            for i in range(OCH):
                _ln_chunk(i)
            for p in range(CCH):
                ps = qkps.tile([P, NO], F32, tag="qk_ps")
                for c in range(CCH):
                    nc.tensor.matmul(ps[:], wqk_sb[:, c, p * P:(p + 1) * P],
                                     xnT[:, c, 0:NO],
                                     start=(c == 0), stop=(c == CCH - 1))
                nc.scalar.activation(qT[:, p, :], ps[:], AF.Identity,
                                     bias=bqk_sb[:, p:p + 1])
            for p in range(CCH):
                _emit_k(p, 0)
            for i in range(OCH, MCH):
                _ln_chunk(i)
            s1_scope.close()
            for p in range(CCH):
                _emit_k(p, 1)

            # QK1 rowmax (exact, [rows, keys] orientation), 2:1 with V.
            # stg col (h*OCH + ncc) holds +max for head h, row-chunk ncc;
            # the sign flips in the tiny stage2 transpose-copy.
            qk1ps = ph_a.enter_context(tc.tile_pool(name="qk1ps", bufs=2, space="PSUM"))
            pstgps = ph_a.enter_context(tc.tile_pool(name="pstgps", bufs=1, space="PSUM"))
            stg = sm1.tile([P, HEADS * OCH], F16, name="stg")

            def _emit_qk1(pp, ncc):
                ps1a = qk1ps.tile([P, NB], F32, tag="ps1")
                ps1b = qk1ps.tile([P, NB], F32, tag="ps1")
                for mh in range(NHALF):
                    nc.tensor.matmul(
                        ps1a[:, mh * 512:(mh + 1) * 512],
                        qT[0:DK, pp, ncc * P:(ncc + 1) * P],
                        kT[0:DK, pp, mh * 512:(mh + 1) * 512],
                        start=True, stop=True)
                    nc.tensor.matmul(
                        ps1b[:, mh * 512:(mh + 1) * 512],
                        qT[DK:P, pp, ncc * P:(ncc + 1) * P],
                        kT[DK:P, pp, mh * 512:(mh + 1) * 512],
                        start=True, stop=True)
                ha, hb = 2 * pp, 2 * pp + 1
                nc.vector.reduce_max(
                    stg[:, ha * OCH + ncc:ha * OCH + ncc + 1], ps1a[:],
                    axis=AX.X)
                nc.vector.reduce_max(
                    stg[:, hb * OCH + ncc:hb * OCH + ncc + 1], ps1b[:],
                    axis=AX.X)

            def _emit_v(dh, mc):
                ps = qkps.tile([P, 512], F32, tag="qk_ps")
                for c in range(CCH):
                    nc.tensor.matmul(
                        ps[:], xnT[:, c, mc * P:(mc + 1) * P],
                        wv_sb[:, c, dh * 512:(dh + 1) * 512],
                        start=(c == 0), stop=(c == CCH - 1))
                # per-head contiguous drains on ScalarE (b_v is folded into
                # b_proj host-side; strided 3D APs on DVE measured 13us!)
                for hh in range(CCH):
                    h = dh * 8 + hh
                    nc.scalar.copy(v_sb[:, mc, h, 0:DK],
                                   ps[:, hh * DK:(hh + 1) * DK])

            for mc in range(MCH):
                nc.gpsimd.memset(v_sb[:, mc, :, DK:DK + 1], 1.0)
            v_iters = [(dh, mc) for dh in range(2) for mc in range(MCH)]
            vi = 0
            for idx, (pp, ncc) in enumerate(
                    (pp, ncc) for pp in range(CCH) for ncc in range(OCH)):
                _emit_qk1(pp, ncc)
                if idx % 2 == 1:
                    _emit_v(*v_iters[vi])
                    vi += 1
            for dh, mc in v_iters[vi:]:
                _emit_v(dh, mc)
            pstg = pstgps.tile([HEADS * OCH, P], F16, name="pstg")
            nc.tensor.transpose(pstg[:], stg[:], ident16[:])
            nc.vector.tensor_scalar(stage2[:], pstg[:], -1.0, None, op0=OP.mult)

        # ================ S3: attention ===================================
        # Per-head [65, ...] K/Q tiles (row 64 = softmax-shift augmentation:
        # ones on K, -rowmax on Q) rotate through 4 manually-managed tiles;
        # their ones rows are set once.  scores(h+1) is emitted before PV(h)
        # so the PE streams through the exp latency.
        with ExitStack() as ph_b:
            kq65 = ph_b.enter_context(tc.tile_pool(name="kq65", bufs=1))
            att = ph_b.enter_context(tc.tile_pool(name="att", bufs=2))
            sm = ph_b.enter_context(tc.tile_pool(name="sm", bufs=4))
            rbc = ph_b.enter_context(tc.tile_pool(name="rbc", bufs=3))
            drp = ph_b.enter_context(tc.tile_pool(name="drp", bufs=2, space="DRAM"))
            qk2ps = ph_b.enter_context(
                tc.tile_pool(name="qk2ps", bufs=2, space="PSUM"))
            ctxps = ph_b.enter_context(
                tc.tile_pool(name="ctxps", bufs=4, space="PSUM"))
            NKQ = 4
            kts = [kq65.tile([DK + 1, NB], F16, name=f"kt65_{i}")
                   for i in range(NKQ)]
            qts = [kq65.tile([DK + 1, NO], F16, name=f"qt65_{i}")
                   for i in range(NKQ)]
            for i in range(NKQ):
                nc.gpsimd.memset(kts[i][DK:DK + 1, :], 1.0)


            pend = {}

            def _emit_scores(h):
                half, pp, off = h // CCH, h // 2, (h % 2) * DK
                kt, qt = kts[h % NKQ], qts[h % NKQ]
                nc.vector.tensor_copy(kt[0:DK, :], kT[off:off + DK, pp, :])
                nc.vector.tensor_copy(qt[0:DK, :], qT[off:off + DK, pp, :])
                nc.sync.dma_start(qt[DK:DK + 1, :],
                                  stage2[h * OCH:(h + 1) * OCH, :])
                attnT = att.tile([P, MCH, 512], BF16, tag="attnT")
                for mg in range(MCH // 2):
                    ps2 = qk2ps.tile([P, 2, 512], F32, tag="ps2")
                    for j in range(2):
                        mc = mg * 2 + j
                        nc.tensor.matmul(
                            ps2[:, j, :], kt[:, mc * P:(mc + 1) * P], qt[:],
                            start=True, stop=True)
                    nc.scalar.activation(
                        attnT[:, mg * 2:mg * 2 + 2, :], ps2[:], AF.Exp)
                return attnT

            def _emit_pv(h, attnT):
                ps3 = ctxps.tile([DK + 1, 512], F32, tag="ps3")
                for mc in range(MCH):
                    nc.tensor.matmul(ps3[:], v_sb[:, mc, h, :], attnT[:, mc, :],
                                     start=(mc == 0), stop=(mc == MCH - 1))
                s_row = sm.tile([1, 512], F32, tag="s_row")
                nc.vector.tensor_copy(s_row[:], ps3[DK:DK + 1, :])
                s_scr = drp.tile([1, 512], F32, tag="s_scr")
                nc.sync.dma_start(s_scr[:], s_row[:])
                s_sq = sm.tile([P, 4], F32, tag="s_sq")
                nc.sync.dma_start(
                    s_sq[:],
                    bass.AP(tensor=s_scr.tensor, offset=s_scr.offset,
                            ap=[[4, P], [1, 4]]))
                r_sq = sm.tile([P, 4], F32, tag="r_sq")
                nc.vector.reciprocal(r_sq[:], s_sq[:])
                r_scr = drp.tile([1, 512], F32, tag="r_scr")
                nc.sync.dma_start(
                    bass.AP(tensor=r_scr.tensor, offset=r_scr.offset,
                            ap=[[4, P], [1, 4]]), r_sq[:])
                r_bc = rbc.tile([DK, 512], F32, tag="r_bc")
                nc.sync.dma_start(
                    r_bc[:],
                    bass.AP(tensor=r_scr.tensor, offset=r_scr.offset,
                            ap=[[0, DK], [1, 512]]))
                pend[h] = (ps3, r_bc)

            def _emit_norm(h):
                off2 = (h % 2) * DK
                ps3h, r_bch = pend.pop(h)
                nc.vector.tensor_tensor(
                    out=ctxT[off2:off2 + DK, h // 2, :], in0=ps3h[0:DK, :],
                    in1=r_bch[:], op=OP.mult)

            prev_at = None
            for h in range(HEADS):
                at = _emit_scores(h)
                if h >= 1:
                    _emit_pv(h - 1, prev_at)
                if h >= 2:
                    _emit_norm(h - 2)
                if h == 3:
                    # proj weight prefetch, queued behind the first heads'
                    # denominator bounces
                    for c in range(CCH):
                        nc.sync.dma_start(wp_sb[:, c, :],
                                          w_proj[c * P:(c + 1) * P, :])
                prev_at = at
            _emit_pv(HEADS - 1, prev_at)
            _emit_norm(HEADS - 2)
            _emit_norm(HEADS - 1)

        # ================ S4: proj + residual =============================
        with ExitStack() as ph_c:
            psb_pool = ph_c.enter_context(tc.tile_pool(name="psb", bufs=1))
            p_sb = psb_pool.tile([P, OCH, C], BF16)
            pps = ph_c.enter_context(tc.tile_pool(name="pps", bufs=2, space="PSUM"))
            tps2 = ph_c.enter_context(tc.tile_pool(name="tps2", bufs=4, space="PSUM"))
            sc = ph_c.enter_context(tc.tile_pool(name="sc", bufs=3))
            for co in range(CCH):
                ps = pps.tile([P, 512], F32)
                for c in range(CCH):
                    nc.tensor.matmul(ps[:], wp_sb[:, c, co * P:(co + 1) * P],
                                     ctxT[:, c, :],
                                     start=(c == 0), stop=(c == CCH - 1))
                pT_i = sc.tile([P, 512], BF16, tag="pT_i")
                nc.scalar.activation(pT_i[:], ps[:], AF.Identity,
                                     bias=bproj_sb[:, co:co + 1])
                for ncc in range(OCH):
                    pst = tps2.tile([P, P], BF16)
                    nc.tensor.transpose(
                        pst[:], pT_i[:, ncc * P:(ncc + 1) * P], identbf[:])
                    nc.vector.tensor_copy(p_sb[:, ncc, co * P:(co + 1) * P], pst[:])
            for i in range(OCH):
                nc.vector.tensor_tensor(
                    out=x2[:, i, :], in0=x_own[:, i, :], in1=p_sb[:, i, :], op=OP.add)

    # ================ S4b: LN2 + transpose ================================
    with ExitStack() as mlp_scope:
        mlp = mlp_scope.enter_context(tc.tile_pool(name="mlp", bufs=1))
        x2nT = mlp.tile([P, CCH, NO], BF16)
        hT = mlp.tile([P, FFCH, NO], BF16)
        wf2 = mlp_scope.enter_context(tc.tile_pool(name="wf2", bufs=1))
        wf2_t = wf2.tile([P, FFCH, C], BF16)
        wf1 = mlp_scope.enter_context(tc.tile_pool(name="wf1", bufs=1))
        wf1_t = wf1.tile([P, CCH, DFF], BF16)
        # wf1 first (fc1 needs every c-chunk before ff=0), wf2 streams
        # behind it and only needs to stay ahead of the fc2 accumulation
        for c in range(CCH):
            nc.sync.dma_start(wf1_t[:, c, :], w_fc1[c * P:(c + 1) * P, :])
        for ff in range(FFCH):
            nc.sync.dma_start(wf2_t[:, ff, :], w_fc2[ff * P:(ff + 1) * P, :])
        with ExitStack() as ph_d:
            ln2 = ph_d.enter_context(tc.tile_pool(name="ln2", bufs=4))
            tps3 = ph_d.enter_context(tc.tile_pool(name="tps3", bufs=4, space="PSUM"))
            for i in range(OCH):
                x2n_i = ln2.tile([P, C], BF16, tag="x2n_i")
                _layer_norm_chunk(nc, ln2, x2[:, i, :], x2n_i, newton=1)
                for c in range(CCH):
                    pst = tps3.tile([P, P], BF16)
                    nc.tensor.transpose(
                        pst[:], x2n_i[:, c * P:(c + 1) * P], identbf[:])
                    nc.vector.tensor_copy(x2nT[:, c, i * P:(i + 1) * P], pst[:])

        # ======== S5/S6: fc1+gelu pipelined into fc2, residual, store =====
        with ExitStack() as ph_f:
            f1ps = ph_f.enter_context(tc.tile_pool(name="f1ps", bufs=2, space="PSUM"))
            f2ps = ph_f.enter_context(tc.tile_pool(name="f2ps", bufs=1, space="PSUM"))
            tps4 = ph_f.enter_context(tc.tile_pool(name="tps4", bufs=2, space="PSUM"))
            sc2 = ph_f.enter_context(tc.tile_pool(name="sc2", bufs=3))
            o_pool = ph_f.enter_context(tc.tile_pool(name="o_pool", bufs=1))
            o_t = o_pool.tile([P, OCH, C], BF16)

            def _fc1(ff):
                ps = f1ps.tile([P, 512], F32)
                for c in range(CCH):
                    nc.tensor.matmul(
                        ps[:], wf1_t[:, c, ff * P:(ff + 1) * P], x2nT[:, c, :],
                        start=(c == 0), stop=(c == CCH - 1))
                nc.scalar.activation(hT[:, ff, :], ps[:], AF.Gelu,
                                     bias=bfc1_sb[:, ff:ff + 1])

            def _fc2(grp, psacc, ff):
                for j in range(4):
                    co = grp * 4 + j
                    nc.tensor.matmul(
                        psacc[j][:], wf2_t[:, ff, co * P:(co + 1) * P],
                        hT[:, ff, :],
                        start=(ff == 0), stop=(ff == FFCH - 1))

            def _fc2_tail(grp, psacc):
                for j in range(4):
                    co = grp * 4 + j
                    oT_i = sc2.tile([P, 512], BF16, tag="oT_i", bufs=2)
                    nc.scalar.activation(oT_i[:], psacc[j][:], AF.Identity,
                                         bias=bfc2_sb[:, co:co + 1])
                    for ncc in range(OCH):
                        pst = tps4.tile([P, P], BF16)
                        nc.tensor.transpose(
                            pst[:], oT_i[:, ncc * P:(ncc + 1) * P], identbf[:])
                        nc.vector.tensor_copy(
                            o_t[:, ncc, co * P:(co + 1) * P], pst[:])

            # group 0 rides along with fc1, one ff behind so the PE never
            # waits on the gelu
            psacc = [f2ps.tile([P, 512], F32, tag=f"f2acc{j}", name=f"f2acc{j}")
                     for j in range(4)]
            for ff in range(FFCH):
                _fc1(ff)
                if ff >= 1:
                    _fc2(0, psacc, ff - 1)
            _fc2(0, psacc, FFCH - 1)
            _fc2_tail(0, psacc)
            psacc = [f2ps.tile([P, 512], F32, tag=f"f2acc{j}", name=f"f2accB{j}")
                     for j in range(4)]
            for ff in range(FFCH):
                _fc2(1, psacc, ff)
            _fc2_tail(1, psacc)

            for i in range(OCH):
                fin = sc2.tile([P, C], F32, tag="fin", bufs=2)
                nc.vector.tensor_tensor(
                    out=fin[:], in0=x2[:, i, :], in1=o_t[:, i, :], op=OP.add)
                nc.sync.dma_start(out[i * P:(i + 1) * P, :], fin[:])


_NC_CACHE = [None]


def _get_nc():
    if _NC_CACHE[0] is None:
        _NC_CACHE[0] = build_program()
    return _NC_CACHE[0]


def _prepare_in_maps(inputs):
    f32 = lambda a: np.ascontiguousarray(np.asarray(a, dtype=np.float32))
    x = f32(inputs["x"])
    g = f32(inputs["norm_g"])
    bb = f32(inputs["norm_b"])
    w_qkv = f32(inputs["w_qkv"])
    b_qkv = f32(inputs["b_qkv"])
    w_proj = f32(inputs["w_proj"])
    b_proj = f32(inputs["b_proj"])
    w_fc1 = f32(inputs["w_fc1"])
    b_fc1 = f32(inputs["b_fc1"])
    w_fc2 = f32(inputs["w_fc2"])
    b_fc2 = f32(inputs["b_fc2"])

    # fold the LN affine into the consuming matmuls; fold the sqrt(dk)
    # score scale into w_q/b_q
    w_qkv_f = w_qkv * g[:, None]
    b_qkv_f = b_qkv + bb @ w_qkv
    scale = float(DK) ** 0.5
    w_q = w_qkv_f[:, 0:C] * scale
    b_q = b_qkv_f[0:C] * scale
    w_k = w_qkv_f[:, C:2 * C]
    b_k = b_qkv_f[C:2 * C]
    w_v = np.ascontiguousarray(w_qkv_f[:, 2 * C:3 * C])
    b_v = np.ascontiguousarray(b_qkv_f[2 * C:3 * C])
    b_proj = b_proj + b_v @ w_proj
    w_fc1_f = w_fc1 * g[:, None]
    b_fc1_f = b_fc1 + bb @ w_fc1

    bf = lambda a: np.ascontiguousarray(a.astype(ml_dtypes.bfloat16))
    f16 = lambda a: np.ascontiguousarray(a.astype(np.float16))
    shared = {
        "w_qk": f16(np.concatenate([w_q, w_k], axis=1)),
        "w_v": f16(w_v),
        "w_proj": bf(w_proj),
        "w_fc1": bf(w_fc1_f),
        "w_fc2": bf(w_fc2),
        "b_qk": np.ascontiguousarray(np.concatenate([b_q, b_k])),
        "b_proj": np.ascontiguousarray(b_proj),
        "b_fc1": np.ascontiguousarray(b_fc1_f),
        "b_fc2": b_fc2,
    }
    in_maps = []
    for core in range(8):
        b, half = core // 2, core % 2
        xb = x[b]
        x_core = np.ascontiguousarray(np.concatenate(
            [xb[half * NO:(half + 1) * NO], xb[(1 - half) * NO:(2 - half) * NO]],
            axis=0))
        in_maps.append({"x": x_core, **shared})
    return in_maps


def kernel(**inputs) -> np.ndarray:
    nc = _get_nc()
    in_maps = _prepare_in_maps(inputs)
    res = run_bass_kernel_spmd(nc, in_maps, list(range(8)))
    out = np.empty((B, N, C), dtype=np.float32)
    for core in range(8):
        b, half = core // 2, core % 2
        out[b, half * NO:(half + 1) * NO] = res.results[core]["out"]
    return out


# revision 29
# speedup vs baseline: 1.7803x; 1.0191x over previous
"""Fused pre-norm transformer block on 8 Trainium2 NeuronCores.

Problem: x[4,1024,1024]; LN -> qkv attention (16 heads) -> proj + residual;
LN -> fc1 -> gelu -> fc2 + residual.  dense_transformer, compute regime.

Sharding (no collectives): 2 cores per batch element, each owning 512 rows.
Each core redundantly computes K/V for its whole batch (key order is
softmax-invariant), so attention, MLP and both residuals are fully
core-local.  The host passes each core its batch's rows with the core's own
512 rows first; outputs are reassembled on host.

Numerics: the QK chain runs in float16 (11-bit mantissa, same as TF32, but
streams the PE at full 2-byte rate where f32r measured ~2-3x slower);
V / attn / proj / MLP run in bf16 with fp32 PSUM accumulation.  LN rsqrt is
refined with Newton steps.  All PE transposes are 16-bit (f32 transposes
measured ~3.5x slower).

Attention layout: scores are computed transposed ([keys, rows]) so exp on
ScalarE writes bf16 attn^T directly in the layout the PV matmul consumes.
The softmax shift (exact per-row max, from a separate QK pass in [rows,
keys] orientation reduced on VectorE) is folded into the scores^T matmul
as a 65th contraction row (ones on the K side, -max on the Q side) keeping
all scores^T matmuls a uniform K=65 fp16 stream.  The shift value is
softmax-invariant so fp16 rounding of the max cancels exactly.  The softmax
denominator comes from an appended ones-column of V; its reciprocal is
broadcast via a DRAM scratch bounce (partition-stride-0 reads are only
legal from DRAM).

Engine balance: LN stats stay on VectorE; PSUM->SBUF transpose drains and
residual-copy casts go to the (otherwise idle) Pool engine; biases and all
activations go to ScalarE, grouped so the 1.3us activation-table reloads
only happen at phase boundaries.  fc1/fc2 are software-pipelined with a
one-ff lag so the PE never waits on the gelu; scores/PV alternate heads
with a one-head lag so the PE never waits on the exp.
"""

import numpy as np
import ml_dtypes
from contextlib import ExitStack

import concourse.bass as bass
import concourse.tile as tile
from concourse import mybir
from concourse.bass_utils import run_bass_kernel_spmd
from concourse.vector_clock import ScopedClock, VectorClock
from concourse.masks import make_identity

F32 = mybir.dt.float32
F16 = mybir.dt.float16
BF16 = mybir.dt.bfloat16
AF = mybir.ActivationFunctionType
OP = mybir.AluOpType
AX = mybir.AxisListType

B, N, C = 4, 1024, 1024
HEADS, DK = 16, 64
DFF = 4096
EPS = 1e-5
NB = 1024   # rows per core's batch (attention keys)
NO = 512    # rows owned per core
P = 128
CCH = C // P      # 8 chunks over C
MCH = NB // P     # 8 key-row chunks
OCH = NO // P     # 4 own-row chunks
FFCH = DFF // P   # 32
NHALF = NB // 512


class SplitDrainTileContext(tile.TileContext):
    """This walrus build rejects >2 sync waits on the tail SP drain
    ("Too many sync wait commands"); split the global-clock waits across
    single-wait drain instructions."""

    def _drain_and_barrier(self, tick_clock, wait_clock):
        nc = self.nc
        gc = tick_clock.global_clock
        n = len(gc)
        for i in range(n):
            if gc[i] > 0:
                vc = VectorClock([0] * n)
                vc.require_at_least(i, gc[i])
                d = nc.sync.drain()
                wait_clock.add_sem_waits(d.ins, ScopedClock({None: vc}))
        nc.sync.drain()
        nc.all_engine_barrier()
        popped = nc._tile_sem_poison_stack.pop()
        assert popped is self._sem_poison
        nc.clear_and_free_semaphores(list(self.sems.allocated().values()))
        nc.all_engine_barrier()


def legalize_waits(nc, cap=1):
    """Walrus here allows at most one sync wait per regular instruction.
    Hoist excess waits onto same-engine NoOps placed just before."""
    n = [0]

    def mknop(engine, wait):
        n[0] += 1
        nop = mybir.InstNoOp(name=f"I-waitfix-{n[0]}", ins=[], outs=[])
        nop.engine = engine
        nop.sync_info = mybir.SyncInfo(on_wait=[wait], on_update=[])
        return nop

    for f in nc.m.functions:
        for bb in f.blocks:
            out = []
            for inst in bb.instructions:
                w = list(inst.sync_info.on_wait or []) if inst.sync_info else []
                if len(w) > cap:
                    for extra in w[:-cap]:
                        out.append(mknop(inst.engine, extra))
                    inst.sync_info.on_wait = w[-cap:]
                out.append(inst)
            bb.instructions = out


def _rsqrt(nc, pool, var, eps, newton=2):
    """[128,1] fp32: 1/sqrt(var+eps); ACT-sqrt seed + Newton steps."""
    a = pool.tile([P, 1], F32, tag="rsq_a")
    nc.vector.tensor_scalar_add(a[:], var, eps)
    s0 = pool.tile([P, 1], F32, tag="rsq_s")
    nc.scalar.activation(s0[:], a[:], AF.Sqrt)
    r = pool.tile([P, 1], F32, tag="rsq_r")
    nc.vector.reciprocal(r[:], s0[:])
    t = pool.tile([P, 1], F32, tag="rsq_t")
    for _ in range(newton):
        nc.vector.tensor_mul(t[:], r[:], r[:])
        nc.vector.tensor_mul(t[:], t[:], a[:])
        nc.vector.tensor_scalar(t[:], t[:], -0.5, 1.5, op0=OP.mult, op1=OP.add)
        nc.vector.tensor_mul(r[:], r[:], t[:])
    return r


def _layer_norm_chunk(nc, pool, x_i, xn_i, newton=2):
    """LN of one [128, C] row-chunk: xn_i = (x - mean(x)) * rsqrt(var+eps)."""
    nsub = C // 512
    stats = pool.tile([P, nsub, 6], F32, tag="ln_stats")
    for s in range(nsub):
        nc.vector.bn_stats(stats[:, s, :], x_i[:, s * 512:(s + 1) * 512])
    mv = pool.tile([P, 2], F32, tag="ln_mv")
    nc.vector.bn_aggr(mv[:], stats[:])
    r = _rsqrt(nc, pool, mv[:, 1:2], EPS, newton=newton)
    nc.vector.tensor_scalar(
        xn_i[:], x_i[:], mv[:, 0:1], r[:], op0=OP.subtract, op1=OP.mult
    )


def build_program(legalize=True):
    nc = bass.Bass()

    x = nc.declare_dram_parameter("x", [NB, C], F32, isOutput=False)
    w_qk = nc.declare_dram_parameter("w_qk", [C, 2 * C], F16, isOutput=False)
    w_v = nc.declare_dram_parameter("w_v", [C, C], F16, isOutput=False)
    w_proj = nc.declare_dram_parameter("w_proj", [C, C], BF16, isOutput=False)
    w_fc1 = nc.declare_dram_parameter("w_fc1", [C, DFF], BF16, isOutput=False)
    w_fc2 = nc.declare_dram_parameter("w_fc2", [DFF, C], BF16, isOutput=False)
    b_qk = nc.declare_dram_parameter("b_qk", [2 * C], F32, isOutput=False)
    b_proj = nc.declare_dram_parameter("b_proj", [C], F32, isOutput=False)
    b_fc1 = nc.declare_dram_parameter("b_fc1", [DFF], F32, isOutput=False)
    b_fc2 = nc.declare_dram_parameter("b_fc2", [C], F32, isOutput=False)
    out = nc.declare_dram_parameter("out", [NO, C], F32, isOutput=True)

    with SplitDrainTileContext(nc) as tc:
        with ExitStack() as ctx:
            _build_body(
                nc, tc, ctx,
                x, w_qk, w_v, w_proj, w_fc1, w_fc2,
                b_qk, b_proj, b_fc1, b_fc2, out,
            )
    if legalize:
        legalize_waits(nc)
    return nc


def _build_body(nc, tc, ctx, x, w_qk, w_v, w_proj, w_fc1, w_fc2,
                b_qk, b_proj, b_fc1, b_fc2, out):
    perm = ctx.enter_context(tc.tile_pool(name="perm", bufs=1))
    small = ctx.enter_context(tc.tile_pool(name="small", bufs=3))

    # --- constants / biases ------------------------------------------------
    ident16 = perm.tile([P, P], F16)
    make_identity(nc, ident16[:])
    identbf = perm.tile([P, P], BF16)
    make_identity(nc, identbf[:])

    bqk_sb = perm.tile([P, 2 * CCH], F32)
    nc.sync.dma_start(bqk_sb[:], b_qk.rearrange("(c p) -> p c", p=P))
    bproj_sb = perm.tile([P, CCH], F32)
    nc.sync.dma_start(bproj_sb[:], b_proj.rearrange("(c p) -> p c", p=P))
    bfc1_sb = perm.tile([P, FFCH], F32)
    nc.sync.dma_start(bfc1_sb[:], b_fc1.rearrange("(c p) -> p c", p=P))
    bfc2_sb = perm.tile([P, CCH], F32)
    nc.sync.dma_start(bfc2_sb[:], b_fc2.rearrange("(c p) -> p c", p=P))
    x_own = perm.tile([P, OCH, C], BF16)      # own rows (residual 1)
    x2 = perm.tile([P, OCH, C], BF16)         # post-attn residual stream

    with ExitStack() as kqv_scope:
        kqv = kqv_scope.enter_context(tc.tile_pool(name="kqv", bufs=1))
        kT = kqv.tile([P, CCH, NB], F16)        # K^T head-pairs [128=2*dk, m]
        qT = kqv.tile([P, CCH, NO], F16)        # Q^T head-pairs [128=2*dk, n_own]
        v_sb = kqv.tile([P, MCH, HEADS, DK + 1], BF16)  # V rows + ones col
        ctxT = kqv.tile([P, CCH, NO], BF16)     # (attn@V)^T, normalized
        stage2 = kqv.tile([HEADS * OCH, P], F16)  # -rowmax per head, transposed
        wp_pool = kqv_scope.enter_context(tc.tile_pool(name="wp", bufs=1))
        wp_sb = wp_pool.tile([P, CCH, C], BF16)

        with ExitStack() as ph_a:
            # ===== S1/S2 software pipeline ================================
            # LN chunks 0-3 -> Q + K(first key half) on the PE while chunks
            # 4-7 LN on VectorE -> K(second half) -> QK1 rowmax 2:1
            # interleaved with V.  The rowmax reduces are VectorE rate-bound,
            # so everything else in that window routes to ScalarE/PE.
            xnt_pool = ph_a.enter_context(tc.tile_pool(name="xnt", bufs=1))
            xnT = xnt_pool.tile([P, CCH, NB], F16)   # LN1(x)^T  [c, n]
            wqv_pool = ph_a.enter_context(tc.tile_pool(name="wqv", bufs=1))
            qkps = ph_a.enter_context(tc.tile_pool(name="qkps", bufs=2, space="PSUM"))
            sm1 = ph_a.enter_context(tc.tile_pool(name="sm1", bufs=2))

            s1_scope = ExitStack()
            ln = s1_scope.enter_context(tc.tile_pool(name="ln", bufs=4))
            xoth = s1_scope.enter_context(tc.tile_pool(name="xoth", bufs=3))
            tps = s1_scope.enter_context(
                tc.tile_pool(name="tps", bufs=4, space="PSUM"))

            # DMA order tracks first use: x chunks 0-3 (LN), w_q (Q),
            # x chunks 4-7, w_k, w_v
            wqk_sb = wqv_pool.tile([P, CCH, 2 * C], F16)
            wv_sb = wqv_pool.tile([P, CCH, C], F16)
            x_tiles = []
            for i in range(MCH):
                x_i = xoth.tile([P, C], F32, tag=f"x_{i}", name=f"x_{i}",
                                bufs=1)
                x_tiles.append(x_i)
            for i in range(OCH):
                nc.sync.dma_start(x_tiles[i][:], x[i * P:(i + 1) * P, :])
            for c in range(CCH):
                nc.sync.dma_start(wqk_sb[:, c, 0:C], w_qk[c * P:(c + 1) * P, 0:C])
            for i in range(OCH, MCH):
                nc.sync.dma_start(x_tiles[i][:], x[i * P:(i + 1) * P, :])
            for c in range(CCH):
                nc.sync.dma_start(wqk_sb[:, c, C:2 * C],
                                  w_qk[c * P:(c + 1) * P, C:2 * C])
            for c in range(CCH):
                nc.sync.dma_start(wv_sb[:, c, :], w_v[c * P:(c + 1) * P, :])

            def _ln_chunk(i):
                x_i = x_tiles[i]
                if i < OCH:
                    nc.vector.tensor_copy(x_own[:, i, :], x_i[:])
                xn_i = xoth.tile([P, C], F16, tag="xn_i")
                _layer_norm_chunk(nc, ln, x_i, xn_i)
                for c in range(CCH):
                    pst = tps.tile([P, P], F16)
                    nc.tensor.transpose(pst[:], xn_i[:, c * P:(c + 1) * P],
                                        ident16[:])
                    nc.vector.tensor_copy(xnT[:, c, i * P:(i + 1) * P],
                                          pst[:])

            def _emit_k(p, nh):
                ps = qkps.tile([P, 512], F32, tag="qk_ps")
                for c in range(CCH):
                    nc.tensor.matmul(
                        ps[:], wqk_sb[:, c, C + p * P:C + (p + 1) * P],
                        xnT[:, c, nh * 512:(nh + 1) * 512],
                        start=(c == 0), stop=(c == CCH - 1))
                nc.scalar.activation(
                    kT[:, p, nh * 512:(nh + 1) * 512], ps[:], AF.Identity,
                    bias=bqk_sb[:, CCH + p:CCH + p + 1])

            for i in range(OCH):
                _ln_chunk(i)
            for p in range(CCH):
                ps = qkps.tile([P, NO], F32, tag="qk_ps")
                for c in range(CCH):
                    nc.tensor.matmul(ps[:], wqk_sb[:, c, p * P:(p + 1) * P],
                                     xnT[:, c, 0:NO],
                                     start=(c == 0), stop=(c == CCH - 1))
                nc.scalar.activation(qT[:, p, :], ps[:], AF.Identity,
                                     bias=bqk_sb[:, p:p + 1])
            for p in range(CCH):
                _emit_k(p, 0)
            for i in range(OCH, MCH):
                _ln_chunk(i)
            s1_scope.close()
            for p in range(CCH):
                _emit_k(p, 1)

            # QK1 rowmax (exact, [rows, keys] orientation), 2:1 with V.
            # stg col (h*OCH + ncc) holds +max for head h, row-chunk ncc;
            # the sign flips in the tiny stage2 transpose-copy.
            qk1ps = ph_a.enter_context(tc.tile_pool(name="qk1ps", bufs=2, space="PSUM"))
            pstgps = ph_a.enter_context(tc.tile_pool(name="pstgps", bufs=1, space="PSUM"))
            stg = sm1.tile([P, HEADS * OCH], F16, name="stg")

            def _emit_qk1(pp, ncc):
                ps1a = qk1ps.tile([P, NB], F32, tag="ps1")
                ps1b = qk1ps.tile([P, NB], F32, tag="ps1")
                for mh in range(NHALF):
                    nc.tensor.matmul(
                        ps1a[:, mh * 512:(mh + 1) * 512],
                        qT[0:DK, pp, ncc * P:(ncc + 1) * P],
                        kT[0:DK, pp, mh * 512:(mh + 1) * 512],
                        start=True, stop=True)
                    nc.tensor.matmul(
                        ps1b[:, mh * 512:(mh + 1) * 512],
                        qT[DK:P, pp, ncc * P:(ncc + 1) * P],
                        kT[DK:P, pp, mh * 512:(mh + 1) * 512],
                        start=True, stop=True)
                ha, hb = 2 * pp, 2 * pp + 1
                nc.vector.reduce_max(
                    stg[:, ha * OCH + ncc:ha * OCH + ncc + 1], ps1a[:],
                    axis=AX.X)
                nc.vector.reduce_max(
                    stg[:, hb * OCH + ncc:hb * OCH + ncc + 1], ps1b[:],
                    axis=AX.X)

            def _emit_v(dh, mc):
                ps = qkps.tile([P, 512], F32, tag="qk_ps")
                for c in range(CCH):
                    nc.tensor.matmul(
                        ps[:], xnT[:, c, mc * P:(mc + 1) * P],
                        wv_sb[:, c, dh * 512:(dh + 1) * 512],
                        start=(c == 0), stop=(c == CCH - 1))
                # per-head contiguous drains on ScalarE (b_v is folded into
                # b_proj host-side; strided 3D APs on DVE measured 13us!)
                for hh in range(CCH):
                    h = dh * 8 + hh
                    nc.scalar.copy(v_sb[:, mc, h, 0:DK],
                                   ps[:, hh * DK:(hh + 1) * DK])

            for mc in range(MCH):
                nc.gpsimd.memset(v_sb[:, mc, :, DK:DK + 1], 1.0)
            HS2 = HEADS * OCH // 2

            def _emit_stage2_half(half):
                pstg = pstgps.tile([HS2, P], F16, tag="pstg", name=f"pstg{half}")
                nc.tensor.transpose(pstg[:], stg[:, half * HS2:(half + 1) * HS2],
                                    ident16[:])
                nc.vector.tensor_scalar(stage2[half * HS2:(half + 1) * HS2, :],
                                        pstg[:], -1.0, None, op0=OP.mult)

            v_iters = [(dh, mc) for dh in range(2) for mc in range(MCH)]
            vi = 0
            for idx, (pp, ncc) in enumerate(
                    (pp, ncc) for pp in range(CCH) for ncc in range(OCH)):
                _emit_qk1(pp, ncc)
                if idx % 2 == 1:
                    _emit_v(*v_iters[vi])
                    vi += 1
                if idx == 15:
                    # heads 0-7 reduced: publish their -rowmax so S3 can
                    # start while heads 8-15 are still reducing
                    _emit_stage2_half(0)
            for dh, mc in v_iters[vi:]:
                _emit_v(dh, mc)
            _emit_stage2_half(1)

        # ================ S3: attention ===================================
        # Per-head [65, ...] K/Q tiles (row 64 = softmax-shift augmentation:
        # ones on K, -rowmax on Q) rotate through 4 manually-managed tiles;
        # their ones rows are set once.  scores(h+1) is emitted before PV(h)
        # so the PE streams through the exp latency.
        with ExitStack() as ph_b:
            kq65 = ph_b.enter_context(tc.tile_pool(name="kq65", bufs=1))
            att = ph_b.enter_context(tc.tile_pool(name="att", bufs=2))
            sm = ph_b.enter_context(tc.tile_pool(name="sm", bufs=4))
            rbc = ph_b.enter_context(tc.tile_pool(name="rbc", bufs=3))
            drp = ph_b.enter_context(tc.tile_pool(name="drp", bufs=2, space="DRAM"))
            qk2ps = ph_b.enter_context(
                tc.tile_pool(name="qk2ps", bufs=2, space="PSUM"))
            ctxps = ph_b.enter_context(
                tc.tile_pool(name="ctxps", bufs=4, space="PSUM"))
            NKQ = 4
            kts = [kq65.tile([DK + 1, NB], F16, name=f"kt65_{i}")
                   for i in range(NKQ)]
            qts = [kq65.tile([DK + 1, NO], F16, name=f"qt65_{i}")
                   for i in range(NKQ)]
            for i in range(NKQ):
                nc.gpsimd.memset(kts[i][DK:DK + 1, :], 1.0)


            pend = {}

            def _load_kq(h):
                pp, off = h // 2, (h % 2) * DK
                kt, qt = kts[h % NKQ], qts[h % NKQ]
                nc.vector.tensor_copy(kt[0:DK, :], kT[off:off + DK, pp, :])
                nc.vector.tensor_copy(qt[0:DK, :], qT[off:off + DK, pp, :])
                nc.sync.dma_start(qt[DK:DK + 1, :],
                                  stage2[h * OCH:(h + 1) * OCH, :])

            # first two heads' K/Q staging jumps the vector queue ahead of
            # nothing here (emitted before the scores), keeping S3 rampup off
            # the critical path
            _load_kq(0)
            _load_kq(1)

            def _emit_scores(h):
                kt, qt = kts[h % NKQ], qts[h % NKQ]
                if h + 2 < HEADS:
                    _load_kq(h + 2)
                attnT = att.tile([P, MCH, 512], BF16, tag="attnT")
                for mg in range(MCH // 2):
                    ps2 = qk2ps.tile([P, 2, 512], F32, tag="ps2")
                    for j in range(2):
                        mc = mg * 2 + j
                        nc.tensor.matmul(
                            ps2[:, j, :], kt[:, mc * P:(mc + 1) * P], qt[:],
                            start=True, stop=True)
                    nc.scalar.activation(
                        attnT[:, mg * 2:mg * 2 + 2, :], ps2[:], AF.Exp)
                return attnT

            def _emit_pv(h, attnT):
                ps3 = ctxps.tile([DK + 1, 512], F32, tag="ps3")
                for mc in range(MCH):
                    nc.tensor.matmul(ps3[:], v_sb[:, mc, h, :], attnT[:, mc, :],
                                     start=(mc == 0), stop=(mc == MCH - 1))
                s_row = sm.tile([1, 512], F32, tag="s_row")
                nc.vector.tensor_copy(s_row[:], ps3[DK:DK + 1, :])
                s_scr = drp.tile([1, 512], F32, tag="s_scr")
                nc.sync.dma_start(s_scr[:], s_row[:])
                s_sq = sm.tile([P, 4], F32, tag="s_sq")
                nc.sync.dma_start(
                    s_sq[:],
                    bass.AP(tensor=s_scr.tensor, offset=s_scr.offset,
                            ap=[[4, P], [1, 4]]))
                r_sq = sm.tile([P, 4], F32, tag="r_sq")
                nc.vector.reciprocal(r_sq[:], s_sq[:])
                r_scr = drp.tile([1, 512], F32, tag="r_scr")
                nc.sync.dma_start(
                    bass.AP(tensor=r_scr.tensor, offset=r_scr.offset,
                            ap=[[4, P], [1, 4]]), r_sq[:])
                r_bc = rbc.tile([DK, 512], F32, tag="r_bc")
                nc.sync.dma_start(
                    r_bc[:],
                    bass.AP(tensor=r_scr.tensor, offset=r_scr.offset,
                            ap=[[0, DK], [1, 512]]))
                pend[h] = (ps3, r_bc)

            def _emit_norm(h):
                off2 = (h % 2) * DK
                ps3h, r_bch = pend.pop(h)
                nc.vector.tensor_tensor(
                    out=ctxT[off2:off2 + DK, h // 2, :], in0=ps3h[0:DK, :],
                    in1=r_bch[:], op=OP.mult)

            prev_at = None
            for h in range(HEADS):
                at = _emit_scores(h)
                if h >= 1:
                    _emit_pv(h - 1, prev_at)
                if h >= 2:
                    _emit_norm(h - 2)
                if h == 3:
                    # proj weight prefetch, queued behind the first heads'
                    # denominator bounces
                    for c in range(CCH):
                        nc.sync.dma_start(wp_sb[:, c, :],
                                          w_proj[c * P:(c + 1) * P, :])
                prev_at = at
            _emit_pv(HEADS - 1, prev_at)
            _emit_norm(HEADS - 2)
            _emit_norm(HEADS - 1)

        # ================ S4: proj + residual =============================
        with ExitStack() as ph_c:
            psb_pool = ph_c.enter_context(tc.tile_pool(name="psb", bufs=1))
            p_sb = psb_pool.tile([P, OCH, C], BF16)
            pps = ph_c.enter_context(tc.tile_pool(name="pps", bufs=2, space="PSUM"))
            tps2 = ph_c.enter_context(tc.tile_pool(name="tps2", bufs=4, space="PSUM"))
            sc = ph_c.enter_context(tc.tile_pool(name="sc", bufs=3))
            for co in range(CCH):
                ps = pps.tile([P, 512], F32)
                for c in range(CCH):
                    nc.tensor.matmul(ps[:], wp_sb[:, c, co * P:(co + 1) * P],
                                     ctxT[:, c, :],
                                     start=(c == 0), stop=(c == CCH - 1))
                pT_i = sc.tile([P, 512], BF16, tag="pT_i")
                nc.scalar.activation(pT_i[:], ps[:], AF.Identity,
                                     bias=bproj_sb[:, co:co + 1])
                for ncc in range(OCH):
                    pst = tps2.tile([P, P], BF16)
                    nc.tensor.transpose(
                        pst[:], pT_i[:, ncc * P:(ncc + 1) * P], identbf[:])
                    nc.vector.tensor_copy(p_sb[:, ncc, co * P:(co + 1) * P], pst[:])
            for i in range(OCH):
                nc.vector.tensor_tensor(
                    out=x2[:, i, :], in0=x_own[:, i, :], in1=p_sb[:, i, :], op=OP.add)

    # ================ S4b: LN2 + transpose ================================
    with ExitStack() as mlp_scope:
        mlp = mlp_scope.enter_context(tc.tile_pool(name="mlp", bufs=1))
        x2nT = mlp.tile([P, CCH, NO], BF16)
        hT = mlp.tile([P, FFCH, NO], BF16)
        wf2 = mlp_scope.enter_context(tc.tile_pool(name="wf2", bufs=1))
        wf2_t = wf2.tile([P, FFCH, C], BF16)
        wf1 = mlp_scope.enter_context(tc.tile_pool(name="wf1", bufs=1))
        wf1_t = wf1.tile([P, CCH, DFF], BF16)
        # wf1 first (fc1 needs every c-chunk before ff=0), wf2 streams
        # behind it and only needs to stay ahead of the fc2 accumulation
        for c in range(CCH):
            nc.sync.dma_start(wf1_t[:, c, :], w_fc1[c * P:(c + 1) * P, :])
        for ff in range(FFCH):
            nc.sync.dma_start(wf2_t[:, ff, :], w_fc2[ff * P:(ff + 1) * P, :])
        with ExitStack() as ph_d:
            ln2 = ph_d.enter_context(tc.tile_pool(name="ln2", bufs=4))
            tps3 = ph_d.enter_context(tc.tile_pool(name="tps3", bufs=4, space="PSUM"))
            for i in range(OCH):
                x2n_i = ln2.tile([P, C], BF16, tag="x2n_i")
                _layer_norm_chunk(nc, ln2, x2[:, i, :], x2n_i, newton=1)
                for c in range(CCH):
                    pst = tps3.tile([P, P], BF16)
                    nc.tensor.transpose(
                        pst[:], x2n_i[:, c * P:(c + 1) * P], identbf[:])
                    nc.vector.tensor_copy(x2nT[:, c, i * P:(i + 1) * P], pst[:])

        # ======== S5/S6: fc1+gelu pipelined into fc2, residual, store =====
        with ExitStack() as ph_f:
            f1ps = ph_f.enter_context(tc.tile_pool(name="f1ps", bufs=2, space="PSUM"))
            f2ps = ph_f.enter_context(tc.tile_pool(name="f2ps", bufs=1, space="PSUM"))
            tps4 = ph_f.enter_context(tc.tile_pool(name="tps4", bufs=2, space="PSUM"))
            sc2 = ph_f.enter_context(tc.tile_pool(name="sc2", bufs=3))
            o_pool = ph_f.enter_context(tc.tile_pool(name="o_pool", bufs=1))
            o_t = o_pool.tile([P, OCH, C], BF16)

            def _fc1(ff):
                ps = f1ps.tile([P, 512], F32)
                for c in range(CCH):
                    nc.tensor.matmul(
                        ps[:], wf1_t[:, c, ff * P:(ff + 1) * P], x2nT[:, c, :],
                        start=(c == 0), stop=(c == CCH - 1))
                nc.scalar.activation(hT[:, ff, :], ps[:], AF.Gelu,
                                     bias=bfc1_sb[:, ff:ff + 1])

            def _fc2(grp, psacc, ff):
                for j in range(4):
                    co = grp * 4 + j
                    nc.tensor.matmul(
                        psacc[j][:], wf2_t[:, ff, co * P:(co + 1) * P],
                        hT[:, ff, :],
                        start=(ff == 0), stop=(ff == FFCH - 1))

            def _fc2_tail(grp, psacc, per_chunk=None):
                oTs = []
                for j in range(4):
                    co = grp * 4 + j
                    oT_i = sc2.tile([P, 512], BF16, tag="oT_i", bufs=4)
                    nc.scalar.activation(oT_i[:], psacc[j][:], AF.Identity,
                                         bias=bfc2_sb[:, co:co + 1])
                    oTs.append(oT_i)
                # ncc-major so each output row-chunk completes (and can be
                # added + stored) as early as possible
                for ncc in range(OCH):
                    for j in range(4):
                        co = grp * 4 + j
                        pst = tps4.tile([P, P], BF16)
                        nc.tensor.transpose(
                            pst[:], oTs[j][:, ncc * P:(ncc + 1) * P], identbf[:])
                        nc.vector.tensor_copy(
                            o_t[:, ncc, co * P:(co + 1) * P], pst[:])
                    if per_chunk is not None:
                        per_chunk(ncc)

            # group 0 rides along with fc1, one ff behind so the PE never
            # waits on the gelu
            psacc = [f2ps.tile([P, 512], F32, tag=f"f2acc{j}", name=f"f2acc{j}")
                     for j in range(4)]
            for ff in range(FFCH):
                _fc1(ff)
                if ff >= 1:
                    _fc2(0, psacc, ff - 1)
            _fc2(0, psacc, FFCH - 1)
            _fc2_tail(0, psacc)
            psacc = [f2ps.tile([P, 512], F32, tag=f"f2acc{j}", name=f"f2accB{j}")
                     for j in range(4)]
            for ff in range(FFCH):
                _fc2(1, psacc, ff)

            def _store_chunk(i):
                fin = sc2.tile([P, C], F32, tag="fin", bufs=2)
                nc.vector.tensor_tensor(
                    out=fin[:], in0=x2[:, i, :], in1=o_t[:, i, :], op=OP.add)
                nc.sync.dma_start(out[i * P:(i + 1) * P, :], fin[:])

            _fc2_tail(1, psacc, per_chunk=_store_chunk)


_NC_CACHE = [None]


def _get_nc():
    if _NC_CACHE[0] is None:
        _NC_CACHE[0] = build_program()
    return _NC_CACHE[0]


def _prepare_in_maps(inputs):
    f32 = lambda a: np.ascontiguousarray(np.asarray(a, dtype=np.float32))
    x = f32(inputs["x"])
    g = f32(inputs["norm_g"])
    bb = f32(inputs["norm_b"])
    w_qkv = f32(inputs["w_qkv"])
    b_qkv = f32(inputs["b_qkv"])
    w_proj = f32(inputs["w_proj"])
    b_proj = f32(inputs["b_proj"])
    w_fc1 = f32(inputs["w_fc1"])
    b_fc1 = f32(inputs["b_fc1"])
    w_fc2 = f32(inputs["w_fc2"])
    b_fc2 = f32(inputs["b_fc2"])

    # fold the LN affine into the consuming matmuls; fold the sqrt(dk)
    # score scale into w_q/b_q
    w_qkv_f = w_qkv * g[:, None]
    b_qkv_f = b_qkv + bb @ w_qkv
    scale = float(DK) ** 0.5
    w_q = w_qkv_f[:, 0:C] * scale
    b_q = b_qkv_f[0:C] * scale
    w_k = w_qkv_f[:, C:2 * C]
    b_k = b_qkv_f[C:2 * C]
    w_v = np.ascontiguousarray(w_qkv_f[:, 2 * C:3 * C])
    b_v = np.ascontiguousarray(b_qkv_f[2 * C:3 * C])
    b_proj = b_proj + b_v @ w_proj
    w_fc1_f = w_fc1 * g[:, None]
    b_fc1_f = b_fc1 + bb @ w_fc1

    bf = lambda a: np.ascontiguousarray(a.astype(ml_dtypes.bfloat16))
    f16 = lambda a: np.ascontiguousarray(a.astype(np.float16))
    shared = {
        "w_qk": f16(np.concatenate([w_q, w_k], axis=1)),
        "w_v": f16(w_v),
        "w_proj": bf(w_proj),
        "w_fc1": bf(w_fc1_f),
        "w_fc2": bf(w_fc2),
        "b_qk": np.ascontiguousarray(np.concatenate([b_q, b_k])),
        "b_proj": np.ascontiguousarray(b_proj),
        "b_fc1": np.ascontiguousarray(b_fc1_f),
        "b_fc2": b_fc2,
    }
    in_maps = []
    for core in range(8):
        b, half = core // 2, core % 2
        xb = x[b]
        x_core = np.ascontiguousarray(np.concatenate(
            [xb[half * NO:(half + 1) * NO], xb[(1 - half) * NO:(2 - half) * NO]],
            axis=0))
        in_maps.append({"x": x_core, **shared})
    return in_maps


def kernel(**inputs) -> np.ndarray:
    nc = _get_nc()
    in_maps = _prepare_in_maps(inputs)
    res = run_bass_kernel_spmd(nc, in_maps, list(range(8)))
    out = np.empty((B, N, C), dtype=np.float32)
    for core in range(8):
        b, half = core // 2, core % 2
        out[b, half * NO:(half + 1) * NO] = res.results[core]["out"]
    return out
